# revision 3
# baseline (speedup 1.0000x reference)
"""KMeans vq_codebook kernel for 8 trn2 NeuronCores.

Data-parallel over N (sharding hint): each core processes 32768 rows of x.

Per-core pipeline (256 tiles of 128 samples, PSUM groups of 2 tiles):
  PE   : h = beta*(2*x@c^T - |c|^2) accumulated in PSUM f32 via two fp16
         passes per tile: (1) xh_tile @ chB, (2) a rank-1 pass ones(2^-6) @
         csqB whose rows 0-63 carry fp16_hi(-beta*csq) and rows 64-127 the
         fp16 residual, injecting csq with ~2^-22 accuracy.
  DVE  : nbh[:, j] = -max_k h (tensor_reduce negate=True, batched per group)
         -- used directly as the Act exp bias and, summed on host, as the
         loss term (loss = sum x^2 + sum(nbh)/beta).
  one-hot (split by tile): Act: oh = exp(h + nbh) -> exactly 1.0 at the
         argmax, ~0 elsewhere (f32-exact bias, no ties); DVE (1 in 8 tiles):
         oh = ((h + nbh) == 0) exact compare, balancing engine load.
  PE   : confusion accumulate, transposed: two matmuls per tile with the oh
         halves as stationary weights and yoh [128, 10] moving; the two
         accumulators live in separate full PSUM banks (2KB zero-region
         rule). Conf matmuls are emitted CONF_DELAY groups late so the
         in-order PE stream never waits on one-hot results.
  out  : nbh [128, 256] f32, conf [128, 20] f32.

Host: fp16 transpose/split of inputs; loss/acc assembly in f64.
"""

import sys

sys.path.insert(0, "/opt/trn_rl_repo")

import numpy as np

import concourse.bass as bass
import concourse.mybir as mybir
import concourse.tile as tile
from concourse.bass_utils import run_bass_kernel_spmd

N_FULL = 262144
D = 128
K = 256
NUM_CORES = 8
NS = N_FULL // NUM_CORES  # 32768 rows per core
NGC = 10  # ground-truth classes

BETA = 128.0
GROUP_B = 2  # tiles per PSUM group (1 bank per group)
PS_BUFS = 5  # PSUM h-group buffers in flight
OH_BUFS = 28
CONF_DELAY = 6  # groups between one-hot production and conf emission
ACT_NUM = 7  # tile j -> Act-exp if (j % ACT_MOD) < ACT_NUM else DVE is_equal
ACT_MOD = 8
SUPERTILE = 4096

F32 = mybir.dt.float32
F16 = mybir.dt.float16

_CACHE = {}


def build_nc(ns=NS, supertile=SUPERTILE, for_sim=False):
    ntiles = ns // 128
    n_super = ns // supertile
    tiles_per_super = supertile // 128
    assert tiles_per_super % GROUP_B == 0

    import concourse.bacc as bacc

    nc = bacc.Bacc("TRN2", target_bir_lowering=False, debug=bool(for_sim))

    xh_d = nc.declare_dram_parameter("xh", [D, ns], F16, isOutput=False)
    chB_d = nc.declare_dram_parameter("chB", [D, K], F16, isOutput=False)
    csqB_d = nc.declare_dram_parameter("csqB", [D, K], F16, isOutput=False)
    ones_d = nc.declare_dram_parameter("ones", [D, 128], F16, isOutput=False)
    yoh_d = nc.declare_dram_parameter("yoh", [128, NGC * ntiles], F16, isOutput=False)
    nbh_out = nc.declare_dram_parameter("nbh", [128, ntiles], F32, isOutput=True)
    conf_out = nc.declare_dram_parameter("conf", [128, 2 * NGC], F32, isOutput=True)

    def is_act_tile(j):
        return (j % ACT_MOD) < ACT_NUM

    with tile.TileContext(nc) as tc:
        with (
            tc.tile_pool(name="const", bufs=1) as constp,
            tc.tile_pool(name="xs", bufs=2) as xsp,
            tc.tile_pool(name="oh", bufs=OH_BUFS) as ohp,
            tc.tile_pool(name="acc", bufs=1) as accp,
            tc.tile_pool(name="ps", bufs=PS_BUFS, space=bass.MemorySpace.PSUM) as psp,
            tc.tile_pool(name="psconf", bufs=1, space=bass.MemorySpace.PSUM) as pscp,
        ):
            chB_t = constp.tile([D, K], F16, tag="chB")
            csqB_t = constp.tile([D, K], F16, tag="csqB")
            ones_t = constp.tile([D, 128], F16, tag="ones")
            yoh_t = constp.tile([128, NGC * ntiles], F16, tag="yoh")
            nc.sync.dma_start(chB_t[:], chB_d[:, :])
            nc.sync.dma_start(csqB_t[:], csqB_d[:, :])
            nc.sync.dma_start(ones_t[:], ones_d[:, :])
            # yoh (0.65 MB, needed only by the delayed conf matmuls) is issued
            # after the first xh chunk so compute starts as early as possible.

            nbh = accp.tile([128, ntiles], F32, tag="nbh")
            # Full 2KB PSUM bank per conf accumulator: a start=True marks the
            # whole zero-region pending-zero, which would discard another
            # group's accumulation living in the same bank.
            conf_a = pscp.tile([128, 512], F32, tag="confA")
            conf_b = pscp.tile([128, 512], F32, tag="confB")

            pending_conf = []

            def emit_conf(entries):
                for j, oh in entries:
                    yslice = yoh_t[:, NGC * j : NGC * (j + 1)]
                    nc.tensor.matmul(
                        conf_a[:, 0:NGC],
                        oh[:, 0:128],
                        yslice,
                        start=(j == 0),
                        stop=(j == ntiles - 1),
                        skip_group_check=True,
                    )
                    nc.tensor.matmul(
                        conf_b[:, 0:NGC],
                        oh[:, 128:256],
                        yslice,
                        start=(j == 0),
                        stop=(j == ntiles - 1),
                        skip_group_check=True,
                    )

            for st in range(n_super):
                xh_s = xsp.tile([D, supertile], F16, tag="xh")
                nc.sync.dma_start(
                    xh_s[:], xh_d[:, st * supertile : (st + 1) * supertile]
                )
                if st == 0:
                    nc.sync.dma_start(yoh_t[:], yoh_d[:, :])
                for g in range(tiles_per_super // GROUP_B):
                    jb = st * tiles_per_super + g * GROUP_B
                    h = psp.tile([128, GROUP_B, K], F32, tag="h")
                    for q in range(GROUP_B):
                        sl = slice(
                            (g * GROUP_B + q) * 128, (g * GROUP_B + q + 1) * 128
                        )
                        nc.tensor.matmul(
                            h[:, q, :], xh_s[:, sl], chB_t[:], start=True, stop=False
                        )
                        nc.tensor.matmul(
                            h[:, q, :], ones_t[:], csqB_t[:], start=False, stop=True
                        )
                    nc.vector.tensor_reduce(
                        nbh[:, jb : jb + GROUP_B],
                        h[:],
                        axis=mybir.AxisListType.X,
                        op=mybir.AluOpType.max,
                        negate=True,
                    )
                    group_entries = []
                    for q in range(GROUP_B):
                        j = jb + q
                        oh = ohp.tile([128, K], F16, tag="oh")
                        if is_act_tile(j):
                            nc.scalar.activation(
                                oh[:],
                                h[:, q, :],
                                mybir.ActivationFunctionType.Exp,
                                bias=nbh[:, j : j + 1],
                                scale=1.0,
                            )
                        else:
                            nc.vector.tensor_scalar(
                                oh[:],
                                h[:, q, :],
                                nbh[:, j : j + 1],
                                0.0,
                                op0=mybir.AluOpType.add,
                                op1=mybir.AluOpType.is_equal,
                            )
                        group_entries.append((j, oh))
                    pending_conf.append(group_entries)
                    if len(pending_conf) > CONF_DELAY:
                        emit_conf(pending_conf.pop(0))
            for entries in pending_conf:
                emit_conf(entries)

            conf_sb = accp.tile([128, 2, NGC], F32, tag="confsb")
            nc.vector.tensor_copy(conf_sb[:, 0, :], conf_a[:, 0:NGC])
            nc.vector.tensor_copy(conf_sb[:, 1, :], conf_b[:, 0:NGC])
            nc.sync.dma_start(nbh_out[:, :], nbh[:])
            nc.sync.dma_start(conf_out[:, :], conf_sb[:])

    nc.compile()
    return nc


def make_host_inputs(x, y, centers, ns=NS, num_cores=NUM_CORES):
    ntiles = ns // 128
    xt = np.ascontiguousarray(x.T).astype(np.float16)  # [128, N]
    chB = (2.0 * BETA * centers.T).astype(np.float16)  # [128, K]
    csq64 = np.sum(centers.astype(np.float64) ** 2, axis=1)
    nb = (-BETA * csq64).astype(np.float32)
    hi = nb.astype(np.float16)
    lo = (nb - hi.astype(np.float32)).astype(np.float16)
    csqB = np.empty((D, K), dtype=np.float16)
    csqB[0:64, :] = hi[None, :]
    csqB[64:128, :] = lo[None, :]
    ones = np.full((D, 128), 2.0**-6, dtype=np.float16)

    y_cores = y.reshape(num_cores, ntiles, 128)
    oh = (y_cores[:, :, :, None] == np.arange(NGC)[None, None, None, :]).astype(
        np.float16
    )
    yoh_all = np.ascontiguousarray(
        oh.transpose(0, 2, 1, 3).reshape(num_cores, 128, ntiles * NGC)
    )

    in_maps = []
    for c in range(num_cores):
        sl = slice(c * ns, (c + 1) * ns)
        in_maps.append(
            {
                "xh": np.ascontiguousarray(xt[:, sl]),
                "chB": chB,
                "csqB": csqB,
                "ones": ones,
                "yoh": yoh_all[c],
            }
        )
    return in_maps


def finalize(x, results, num_cores=NUM_CORES):
    nbh_sum = 0.0
    conf = np.zeros((K, NGC), dtype=np.float64)
    for c in range(num_cores):
        nbh_sum += float(np.asarray(results[c]["nbh"]).astype(np.float64).sum())
        co = np.asarray(results[c]["conf"]).astype(np.float64).reshape(128, 2, NGC)
        conf[0:128] += co[:, 0, :]
        conf[128:256] += co[:, 1, :]

    x64 = x.astype(np.float64)
    x_sq_total = float(np.einsum("nd,nd->", x64, x64, optimize=True))
    loss = np.float32(x_sq_total + nbh_sum / BETA)

    correct_ct = conf.max(axis=1).sum()
    acc = np.float32(correct_ct / np.float32(x.shape[0]))
    return loss, acc


def kernel(x, y, centers):
    x = np.asarray(x, dtype=np.float32)
    y_np = np.asarray(y).astype(np.int64)
    centers = np.asarray(centers, dtype=np.float32)
    n = x.shape[0]
    assert n == N_FULL and x.shape[1] == D and centers.shape == (K, D)

    if "nc" not in _CACHE:
        _CACHE["nc"] = build_nc()
    nc = _CACHE["nc"]

    in_maps = make_host_inputs(x, y_np, centers)
    res_obj = run_bass_kernel_spmd(nc, in_maps, list(range(NUM_CORES)))
    globals()["LAST_EXEC_NS"] = res_obj.exec_time_ns
    return finalize(x, res_obj.results)
